# revision 1
# baseline (speedup 1.0000x reference)
"""MixAdapter (alpha-weighted adapter superposition + joint layernorm + bottleneck MLP).

Strategy (8 NeuronCores):
  Launch A ("merge"): the 80MB adapter stacks are sharded across the 8 cores
    (~10MB each); each core computes its slice of the alpha-weighted merged
    W_down / W_up and the merged LN vectors with fused (W*alpha)+acc DVE ops.
    Host concatenates the 3.2MB of merged params.
  Launch B ("main"): data-parallel over batch (batch elem k -> core k).
    Per core: load x_k^T, LN stats (mean/var over the whole [S,D] slab),
    fold the LN affine into the merged weights (scale W_down^T rows by
    a[d] = rstd*W_ln[d]; bias_h = W_down @ c), down-proj + ReLU(+bias),
    up-proj + residual.  All matmuls accumulate over K-chunks in PSUM.

Everything is fp32; matmuls optionally use the float32r fast path.
"""

import numpy as np

from concourse import bacc, mybir, tile
import concourse.bass as bass
from concourse.bass_utils import run_bass_kernel_spmd

# Problem shapes (hardcoded per the task contract).
B, S, D, BOT, N = 8, 2048, 1024, 400, 25
NCORES = 8
EPS = 1e-5
FP32 = mybir.dt.float32
F32R = mybir.dt.float32r

# d is tiled as d = 128*c + p  (c: chunk 0..7, p: partition 0..127)
DC = D // 128            # 8 d-chunks
OC = (BOT + 127) // 128  # 4 o-chunks (last is a 16-row runt)
SB = S // 512            # 4 s-blocks of 512
O_SZ = [min(128, BOT - 128 * i) for i in range(OC)]  # [128,128,128,16]

USE_F32R = True
MMDT = F32R if USE_F32R else FP32   # dtype of matmul operands in the main kernel
# If True, the residual add reuses the f32r x tiles already in SBUF (x is
# passed as raw fp32 bits typed f32r; any PE-side mantissa truncation only
# affects matmul inputs).  If False, exact x is re-DMAed for the residual.
RESIDUAL_FROM_XT = True

# Per-core slice sizes for the merge launch.
WD_ROWS = BOT // NCORES          # 50 rows of W_down per core
assert WD_ROWS * 1024 == 128 * 400
WU_ROWS = D // NCORES            # 128 rows of W_up per core




# ---------------------------------------------------------------------------
# Launch A: alpha-weighted merge of the adapter stacks (sharded over cores)
# ---------------------------------------------------------------------------

MF = 400 + 400 + 2 * DC  # packed free size per adapter: wd | wu | ln


def build_merge_nc():
    nc = bacc.Bacc("TRN2", target_bir_lowering=False, debug=False,
                   enable_asserts=False, num_devices=NCORES)

    # Per-core input: 1/8 slice of every stack, packed [25, 128, 816]
    # (per adapter: 400 cols of W_down rows, 400 of W_up rows, 16 of LN).
    stack = nc.dram_tensor("stack", [N, 128, MF], FP32, kind="ExternalInput")
    alphas = nc.dram_tensor("alphas", [1, N], FP32, kind="ExternalInput")

    wd_m = nc.dram_tensor("wd_m", [128, 400], FP32, kind="ExternalOutput")
    wu_m = nc.dram_tensor("wu_m", [128, 400], MMDT, kind="ExternalOutput")
    ln_m = nc.dram_tensor("ln_m", [128, 2 * DC], FP32, kind="ExternalOutput")

    with tile.TileContext(nc) as tc:
        with (
            tc.tile_pool(name="consts", bufs=1) as consts,
            tc.tile_pool(name="acc", bufs=1) as accp,
            tc.tile_pool(name="stk", bufs=6) as stk_pool,
            tc.tile_pool(name="psum", bufs=1, space="PSUM") as psum,
        ):
            # Broadcast alphas across partitions: [1,25] -> [128,25] via PE.
            a_sb = consts.tile([1, N], FP32)
            nc.sync.dma_start(a_sb[:], alphas[:])
            ones_row = consts.tile([1, 128], FP32)
            nc.vector.memset(ones_row[:], 1.0)
            pa = psum.tile([128, N], FP32)
            nc.tensor.matmul(pa[:], ones_row[:], a_sb[:], start=True, stop=True)
            a_bc = consts.tile([128, N], FP32)
            nc.scalar.copy(a_bc[:], pa[:])

            acc = accp.tile([128, MF], FP32)
            for n in range(N):
                st = stk_pool.tile([128, MF], FP32)
                nc.sync.dma_start(st[:], stack[n])
                al = a_bc[:, n:n + 1]
                if n == 0:
                    nc.vector.tensor_scalar_mul(acc[:], st[:], al)
                else:
                    nc.vector.scalar_tensor_tensor(
                        acc[:], st[:], al, acc[:],
                        mybir.AluOpType.mult, mybir.AluOpType.add)

            # W_up feeds float32r matmuls in the main launch; round it here so
            # the main kernel can DMA it straight into an f32r tile.
            wu_r = accp.tile([128, 400], MMDT)
            nc.vector.tensor_copy(wu_r[:], acc[:, 400:800])

            nc.sync.dma_start(wd_m[:], acc[:, 0:400])
            nc.sync.dma_start(wu_m[:], wu_r[:])
            nc.sync.dma_start(ln_m[:], acc[:, 800:800 + 2 * DC])

    nc.finalize()
    return nc


# ---------------------------------------------------------------------------
# Launch B: layernorm + down/up projections, one batch element per core
# ---------------------------------------------------------------------------

def build_main_nc():
    nc = bacc.Bacc("TRN2", target_bir_lowering=False, debug=False,
                   enable_asserts=False, num_devices=NCORES)

    # x^T is typed f32r: the host passes the raw fp32 bits and the PE's f32r
    # mode consumes them directly (no on-device rounding pass needed).
    xT = nc.dram_tensor("xT", [D, S], MMDT, kind="ExternalInput")
    wdT = nc.dram_tensor("wdT", [D, BOT], FP32, kind="ExternalInput")   # W_down^T
    # W_up^T arrives pre-rounded to float32r by the merge launch.
    wuT = nc.dram_tensor("wuT", [BOT, D], MMDT, kind="ExternalInput")
    lnm = nc.dram_tensor("lnm", [128, 2 * DC], FP32, kind="ExternalInput")
    yT = nc.dram_tensor("yT", [D, S], FP32, kind="ExternalOutput")

    inv_n = 1.0 / float(S * D)

    with tile.TileContext(nc) as tc:
        with (
            tc.tile_pool(name="xt", bufs=1) as xt_pool,
            tc.tile_pool(name="ht", bufs=1) as ht_pool,
            tc.tile_pool(name="w", bufs=1) as w_pool,
            tc.tile_pool(name="small", bufs=1) as small,
            tc.tile_pool(name="yo", bufs=2) as yo_pool,
            tc.tile_pool(name="pmm", bufs=7, space="PSUM") as pmm,
            tc.tile_pool(name="psc", bufs=1, space="PSUM") as pscp,
        ):
            # ---- weights first: down matmuls must not wait on anything slow --
            wdT_sb = w_pool.tile([128, DC, BOT], FP32)
            nc.sync.dma_start(
                wdT_sb[:], wdT.ap().rearrange("(c p) o -> p c o", p=128))
            lnm_sb = small.tile([128, 2 * DC], FP32)
            nc.sync.dma_start(lnm_sb[:], lnm[:])

            # Fold W_ln (known before stats!) into the down weights:
            #   wdTw[d,o] = W_ln[d] * wdT[d,o]      (f32r stationary operand)
            # The remaining LN pieces are scalars/per-o vectors applied at ReLU:
            #   h = relu( rstd * (wdTw^T @ x) + bias_h )
            wdTw_sb = w_pool.tile([128, DC, BOT], MMDT, tag="wdTw")
            for c in range(DC):
                nc.scalar.activation(wdTw_sb[:, c, :], wdT_sb[:, c, :],
                                     mybir.ActivationFunctionType.Copy,
                                     scale=lnm_sb[:, c:c + 1])

            # ---- stream x in (already f32r-typed), stats on the fly ----
            sums = small.tile([128, DC], FP32)
            sqs = small.tile([128, DC], FP32)
            sq_scratch = small.tile([128, S], FP32)
            xt = []
            xtf = []  # fp32 bit view for DVE/ACT consumers
            for c in range(DC):
                t = xt_pool.tile([128, S], MMDT, name=f"xt{c}", tag=f"xt{c}")
                nc.sync.dma_start(t[:], xT[128 * c:128 * (c + 1), :])
                tf = t[:].bitcast(FP32)
                nc.vector.tensor_reduce(sums[:, c:c + 1], tf,
                                        mybir.AxisListType.X, mybir.AluOpType.add)
                nc.scalar.activation(sq_scratch[:], tf,
                                     mybir.ActivationFunctionType.Square,
                                     accum_out=sqs[:, c:c + 1])
                xt.append(t)
                xtf.append(tf)

            # W_up^T loaded after x so the x stream owns the DMA bandwidth
            # early (wuT is not needed until the up-projection).
            wuT_sb = []
            for oc in range(OC):
                t = w_pool.tile([128, D], MMDT, tag=f"wuT{oc}")
                nc.sync.dma_start(t[:O_SZ[oc], :], wuT[128 * oc:128 * oc + O_SZ[oc], :])
                wuT_sb.append(t)

            s1 = small.tile([128, 1], FP32)
            s2 = small.tile([128, 1], FP32)
            nc.vector.tensor_reduce(s1[:], sums[:], mybir.AxisListType.X,
                                    mybir.AluOpType.add)
            nc.vector.tensor_reduce(s2[:], sqs[:], mybir.AxisListType.X,
                                    mybir.AluOpType.add)

            ones_col = small.tile([128, 1], FP32)
            nc.vector.memset(ones_col[:], 1.0)
            ones_row = small.tile([1, 128], FP32)
            nc.vector.memset(ones_row[:], 1.0)

            # One PSUM bank for every scalar-sized matmul output:
            #   cols 0:2 -> [1,1] partition sums; 2:4 -> [128,1] broadcasts;
            #   cols 4:8 -> bias_h per o-tile.
            psc = pscp.tile([128, 8], FP32)
            nc.tensor.matmul(psc[0:1, 0:1], ones_col[:], s1[:], start=True, stop=True)
            nc.tensor.matmul(psc[0:1, 1:2], ones_col[:], s2[:], start=True, stop=True)

            sc = small.tile([1, 8], FP32)  # mu, e2, musq, var, std, rstd, negmu
            mu, e2, musq, var, std, rstd, negmu = (sc[:, i:i + 1] for i in range(7))
            eps_sb = small.tile([1, 1], FP32)
            nc.vector.memset(eps_sb[:], EPS)
            nc.vector.tensor_scalar_mul(mu, psc[0:1, 0:1], inv_n)
            nc.vector.tensor_scalar_mul(e2, psc[0:1, 1:2], inv_n)
            nc.vector.tensor_tensor(musq, mu, mu, mybir.AluOpType.mult)
            nc.vector.tensor_tensor(var, e2, musq, mybir.AluOpType.subtract)
            nc.scalar.activation(std, var, mybir.ActivationFunctionType.Sqrt,
                                 bias=eps_sb[:])
            nc.vector.reciprocal(rstd, std)
            nc.vector.tensor_scalar_mul(negmu, mu, -1.0)

            # broadcast rstd / negmu across partitions
            nc.tensor.matmul(psc[:, 2:3], ones_row[:], rstd, start=True, stop=True)
            nc.tensor.matmul(psc[:, 3:4], ones_row[:], negmu, start=True, stop=True)
            bc = small.tile([128, 2], FP32)
            nc.scalar.copy(bc[:], psc[:, 2:4])

            # a[d] = rstd * W_ln_m[d] ; cvec[d] = b_ln_m[d] - mu * a[d]
            a_sb = small.tile([128, DC], FP32)
            c_sb = small.tile([128, DC], FP32)
            nc.vector.tensor_scalar_mul(a_sb[:], lnm_sb[:, 0:DC], bc[:, 0:1])
            nc.vector.scalar_tensor_tensor(
                c_sb[:], a_sb[:], bc[:, 1:2], lnm_sb[:, DC:2 * DC],
                mybir.AluOpType.mult, mybir.AluOpType.add)

            # bias_h[o] = sum_d wdT[d,o] * cvec[d]   (small PE matmuls)
            bias_sb = small.tile([128, OC], FP32)
            for ot in range(OC):
                osz = O_SZ[ot]
                for c in range(DC):
                    nc.tensor.matmul(psc[:osz, 4 + ot:5 + ot],
                                     wdT_sb[:, c, 128 * ot:128 * ot + osz],
                                     c_sb[:, c:c + 1],
                                     start=(c == 0), stop=(c == DC - 1))
                nc.scalar.copy(bias_sb[:osz, ot:ot + 1], psc[:osz, 4 + ot:5 + ot])

            # ---- down-proj: hT = relu( rstd * (wdTw^T @ xT) + bias_h ) ------
            ht = [ht_pool.tile([128, S], MMDT, name=f"ht{ot}", tag=f"ht{ot}")
                  for ot in range(OC)]
            for ot in range(OC):
                osz = O_SZ[ot]
                phs = [pmm.tile([128, 512], FP32, name=f"ph{ot}_{sb}", tag="mm")
                       for sb in range(SB)]
                for c in range(DC):
                    lhsT = wdTw_sb[:, c, 128 * ot:128 * ot + osz]
                    for sb in range(SB):
                        nc.tensor.matmul(
                            phs[sb][:osz, :], lhsT,
                            xt[c][:, 512 * sb:512 * (sb + 1)],
                            start=(c == 0), stop=(c == DC - 1))
                for sb in range(SB):
                    nc.scalar.activation(
                        ht[ot][:osz, 512 * sb:512 * (sb + 1)], phs[sb][:osz, :],
                        mybir.ActivationFunctionType.Relu,
                        bias=bias_sb[:osz, ot:ot + 1], scale=bc[:osz, 0:1])

            # ---- up-proj + residual: yT[d,s] = xT[d,s] + sum_o wuT[o,d]*hT[o,s]
            # (the xt tiles hold the exact x bits — DMA does not round — so the
            #  residual is exact; one batched store per d-chunk)
            for c in range(DC):
                pys = [pmm.tile([128, 512], FP32, name=f"py{c}_{sb}", tag="mm")
                       for sb in range(SB)]
                for oc in range(OC):
                    osz = O_SZ[oc]
                    lhsT = wuT_sb[oc][:osz, 128 * c:128 * (c + 1)]
                    for sb in range(SB):
                        nc.tensor.matmul(
                            pys[sb][:], lhsT,
                            ht[oc][:osz, 512 * sb:512 * (sb + 1)],
                            start=(oc == 0), stop=(oc == OC - 1))
                yo = yo_pool.tile([128, S], FP32, name=f"yo{c}", tag="yo")
                for sb in range(SB):
                    nc.vector.tensor_tensor(
                        yo[:, 512 * sb:512 * (sb + 1)], pys[sb][:],
                        xtf[c][:, 512 * sb:512 * (sb + 1)], mybir.AluOpType.add)
                nc.sync.dma_start(yT[128 * c:128 * (c + 1), :], yo[:])

    nc.finalize()
    return nc


# ---------------------------------------------------------------------------
# Host-side orchestration
# ---------------------------------------------------------------------------

def prep_merge_inputs(alphas, W_down_all, W_up_all, W_ln_all, b_ln_all):
    """Build the 8 per-core input maps for the merge launch."""
    a_in = np.ascontiguousarray(alphas.reshape(1, N)).astype(np.float32)
    # ln block[n] = [W_ln chunks^T | b_ln chunks^T] -> [128, 16]; d = 128*c + p
    wln = W_ln_all.reshape(N, DC, 128).transpose(0, 2, 1)   # [N,128,8]
    bln = b_ln_all.reshape(N, DC, 128).transpose(0, 2, 1)
    ln_blk = np.concatenate([wln, bln], axis=2)             # [N,128,16]
    in_maps = []
    for k in range(NCORES):
        wd_k = W_down_all[:, WD_ROWS * k:WD_ROWS * (k + 1), :].reshape(N, 128, 400)
        wu_k = W_up_all[:, WU_ROWS * k:WU_ROWS * (k + 1), :]  # [N,128,400]
        stack = np.ascontiguousarray(
            np.concatenate([wd_k, wu_k, ln_blk], axis=2))     # [N,128,816]
        in_maps.append({"stack": stack, "alphas": a_in})
    return in_maps


def assemble_merge(results):
    """Per-core merge slices -> wdT [D,BOT], wuT [BOT,D], ln_m [128,16]."""
    W_down = np.concatenate(
        [results[k]["wd_m"].reshape(WD_ROWS, D) for k in range(NCORES)], axis=0)
    W_up = np.concatenate([results[k]["wu_m"] for k in range(NCORES)], axis=0)
    wdT = np.ascontiguousarray(W_down.T)          # [D, BOT]
    wuT = np.ascontiguousarray(W_up.T)            # W_up is [D,BOT] -> wuT [BOT, D]
    ln_m = results[0]["ln_m"]
    return wdT, wuT, ln_m


def prep_main_inputs(x, wdT, wuT, ln_m):
    return [{
        "xT": np.ascontiguousarray(x[k].T),
        "wdT": wdT, "wuT": wuT, "lnm": ln_m,
    } for k in range(NCORES)]


_NC_CACHE = {}


def _get_nc(which):
    if which not in _NC_CACHE:
        _NC_CACHE[which] = build_merge_nc() if which == "merge" else build_main_nc()
    return _NC_CACHE[which]


def run(inputs, trace=False, trace_cores=None):
    """Run the full pipeline; returns (output, results_A, results_B)."""
    core_ids = list(range(NCORES))
    nc_a = _get_nc("merge")
    in_a = prep_merge_inputs(inputs["alphas"], inputs["W_down_all"],
                             inputs["W_up_all"], inputs["W_ln_all"],
                             inputs["b_ln_all"])
    res_a = run_bass_kernel_spmd(nc_a, in_a, core_ids=core_ids, trace=trace,
                                 trace_cores=trace_cores)
    wdT, wuT, ln_m = assemble_merge(res_a.results)

    nc_b = _get_nc("main")
    in_b = prep_main_inputs(inputs["x"], wdT, wuT, ln_m)
    res_b = run_bass_kernel_spmd(nc_b, in_b, core_ids=core_ids, trace=trace,
                                 trace_cores=trace_cores)
    out = np.stack([res_b.results[k]["yT"].T for k in range(NCORES)], axis=0)
    return np.ascontiguousarray(out).astype(np.float32), res_a, res_b


def kernel(**inputs):
    inputs = {k: np.asarray(v, dtype=np.float32) for k, v in inputs.items()}
    out, _, _ = run(inputs)
    return out

